# revision 1
# baseline (speedup 1.0000x reference)
"""Trainium2 Bass kernel for a 2-layer encoder-decoder LSTM.

Problem: x [512, 256, 1] -> encoder 2-layer LSTM (H=512) -> autoregressive
decoder (64 steps, head feedback) -> out [512, 64].

Strategy: data-parallel across 8 NeuronCores (batch 512 -> 64 per core), all
weights replicated and SBUF-resident.  Per core each timestep needs 3
matmuls of [64,512] @ [512,2048], run "activation-stationary" (lhsT = h.T
chunk [128,64], moving = W.T [128,512] slices).  Each "pair" step computes
layer-1 @ t together with layer-0 @ t+1 so the PE always has a deep stream
of independent work.

Two dtype modes (LSTM_MMDT):
 - bfloat16: PE column tiling packs the two cells onto separate column
   groups (layer-1 -> PSUM partitions 0..63, layer-0 -> 64..127) which run
   concurrently, and the activation/vector engines process both cells
   stacked [128, *] at full width.
 - float32r: tf32-like full-width mode (the PE uses both columns of each
   column pair, so no column tiling is possible); the two cells use
   separate PSUM tiles on partitions 0..63.

Biases and the scalar input term are folded into extra K=1/K=2 matmul
accumulation passes.  h is re-transposed each step with PE transpose; the
transposed h.T doubles as the moving operand of the decoder head matmul.
"""

import os
import sys
import time

import numpy as np

B_FULL, T, H, HORIZON = 512, 256, 512, 64
NCORES = 8
B = B_FULL // NCORES          # 64 batch rows per core
G = 4 * H                     # 2048 gate columns
KC = H // 128                 # 4 contraction chunks
NCH = G // 512                # 4 output chunks of 512 (one PSUM bank each)

# gate chunk indices (PyTorch order: i, f, g, o)
I_SL, F_SL, G_SL, O_SL = 0, 1, 2, 3

_CACHE = {}
LAST_EXEC_NS = None
LAST_RESULTS = None


def _build(n_enc=T, n_dec=HORIZON, mm_dt="float32r"):
    """Build the Bass module (single SPMD program, run on all 8 cores)."""
    from contextlib import ExitStack

    import concourse.mybir as mybir
    import concourse.tile as tile
    from concourse import bacc
    from concourse.masks import make_identity

    dt = mybir.dt
    MDT = getattr(dt, mm_dt)
    F32 = dt.float32
    AF = mybir.ActivationFunctionType
    NT = n_enc + n_dec            # total timesteps
    PAIRED = mm_dt != "float32r"  # col-tiled two-group mode

    nc = bacc.Bacc("TRN2", target_bir_lowering=False, debug=False)

    # ---------------- DRAM parameters (per-core views) ----------------
    xaug = nc.declare_dram_parameter("xaug", [n_enc + 1, 2, B], MDT, isOutput=False)
    wt = {}
    for nm in ("e0", "e1i", "e1h", "d0", "d1i", "d1h"):
        wt[nm] = nc.declare_dram_parameter(f"wt_{nm}", [128, KC, G], MDT, isOutput=False)
    rows_e0 = nc.declare_dram_parameter("rows_e0", [2, G], MDT, isOutput=False)
    rows_e1 = nc.declare_dram_parameter("rows_e1", [1, G], MDT, isOutput=False)
    rows_d0 = nc.declare_dram_parameter("rows_d0", [2, G], MDT, isOutput=False)
    rows_d1 = nc.declare_dram_parameter("rows_d1", [1, G], MDT, isOutput=False)
    headt_d = nc.declare_dram_parameter("headt", [128, KC], MDT, isOutput=False)
    headb_d = nc.declare_dram_parameter("headb", [1, B], MDT, isOutput=False)
    zeros_d = nc.declare_dram_parameter("zeros", [128, KC * B], MDT, isOutput=False)
    outT = nc.declare_dram_parameter("outT", [n_dec, B], F32, isOutput=True)

    with ExitStack() as ctx:
        tc = ctx.enter_context(tile.TileContext(nc))
        wpool = ctx.enter_context(tc.tile_pool(name="w", bufs=1))
        consts = ctx.enter_context(tc.tile_pool(name="consts", bufs=1))
        states = ctx.enter_context(tc.tile_pool(name="states", bufs=2))
        acts = ctx.enter_context(tc.tile_pool(name="acts", bufs=2))
        xpool = ctx.enter_context(tc.tile_pool(name="xp", bufs=4))
        gpool = ctx.enter_context(
            tc.tile_pool(name="gp", bufs=6, space="PSUM"))
        tpool = ctx.enter_context(tc.tile_pool(name="tp", bufs=2, space="PSUM"))

        # ---------------- constants ----------------
        ident = consts.tile([128, 128], F32, tag="ident")
        make_identity(nc, ident)
        ones64 = consts.tile([1, B], MDT, tag="ones64")
        nc.sync.dma_start(out=ones64, in_=xaug[0, 1:2, :])
        headt = consts.tile([128, KC], MDT, tag="headt")
        nc.sync.dma_start(out=headt, in_=headt_d[:, :])
        headb = consts.tile([1, B], MDT, tag="headb")
        nc.sync.dma_start(out=headb, in_=headb_d[:, :])
        dec_stage = consts.tile([2, B], MDT, tag="dec_stage")
        # (x_last, ones): row 1 stays 1.0 forever; row 0 overwritten per step
        nc.sync.dma_start(out=dec_stage, in_=xaug[n_enc, :, :])

        # weight tiles: encoder set now, decoder set later (same tags)
        def load_wset(phase):
            p = "e" if phase == 0 else "d"
            w0 = wpool.tile([128, KC, G], MDT, tag="w0")
            nc.sync.dma_start(out=w0, in_=wt[p + "0"][:, :, :])
            w1i = wpool.tile([128, KC, G], MDT, tag="w1i")
            nc.sync.dma_start(out=w1i, in_=wt[p + "1i"][:, :, :])
            w1h = wpool.tile([128, KC, G], MDT, tag="w1h")
            nc.sync.dma_start(out=w1h, in_=wt[p + "1h"][:, :, :])
            r0 = wpool.tile([2, G], MDT, tag="rows0")
            nc.sync.dma_start(out=r0, in_=(rows_e0 if phase == 0 else rows_d0)[:, :])
            r1 = wpool.tile([1, G], MDT, tag="rows1")
            nc.sync.dma_start(out=r1, in_=(rows_e1 if phase == 0 else rows_d1)[:, :])
            return dict(w0=w0, w1i=w1i, w1h=w1h, r0=r0, r1=r1)

        enc_w = load_wset(0)
        dec_w = None

        TOP = slice(0, 64)
        BOT = slice(64, 128)

        # ---------------- initial state ----------------
        h0T = None                       # [128, KC*B] transposed h0 (set by prologue)
        h1T = states.tile([128, KC * B], MDT, tag="h1T")
        nc.sync.dma_start(out=h1T, in_=zeros_d[:, :])
        if PAIRED:
            c_prev = states.tile([128, 512], F32, tag="c")
            nc.vector.memset(c_prev, 0.0)
            c1_prev = c0_prev = None
        else:
            c1_prev = states.tile([64, 512], F32, tag="c1")
            nc.vector.memset(c1_prev, 0.0)
            c0_prev = states.tile([64, 512], F32, tag="c0")
            nc.vector.memset(c0_prev, 0.0)
            c_prev = None

        def alloc_tset(sfx):
            return dict(
                ifsb=acts.tile([128, 1024], F32, tag="ifsb" + sfx, name="ifsb" + sfx),
                gsb=acts.tile([128, 512], F32, tag="gsb" + sfx, name="gsb" + sfx),
                osb=acts.tile([128, 512], F32, tag="osb" + sfx, name="osb" + sfx),
                t1=acts.tile([128, 512], F32, tag="t1" + sfx, name="t1" + sfx),
                t2=acts.tile([128, 512], F32, tag="t2" + sfx, name="t2" + sfx),
                tcsb=acts.tile([128, 512], F32, tag="tcsb" + sfx, name="tcsb" + sfx),
                hsb=acts.tile([128, 512], F32, tag="hsb" + sfx, name="hsb" + sfx),
            )

        def emit_cell(gps, gsl, ts, sl, c_prev_ap, c_new_ap):
            """One LSTM cell's activations + state update.
            gps: 4 psum chunk tiles; gsl: partition slice in psum;
            ts: act tile set; sl: partition slice in act tiles."""
            nc.scalar.activation(ts["ifsb"][sl, 0:512], gps[I_SL][gsl, :], AF.Sigmoid)
            nc.scalar.activation(ts["ifsb"][sl, 512:1024], gps[F_SL][gsl, :], AF.Sigmoid)
            nc.scalar.activation(ts["gsb"][sl, :], gps[G_SL][gsl, :], AF.Tanh)
            nc.vector.tensor_mul(ts["t1"][sl, :], ts["ifsb"][sl, 512:1024], c_prev_ap)
            nc.vector.tensor_mul(ts["t2"][sl, :], ts["ifsb"][sl, 0:512], ts["gsb"][sl, :])
            nc.vector.tensor_add(c_new_ap, ts["t1"][sl, :], ts["t2"][sl, :])
            # tanh(c) before sigmoid(o) in ACT program order: o depends on the
            # last-finishing gate chunk, tanh(c) only on i/f/g
            nc.scalar.activation(ts["tcsb"][sl, :], c_new_ap, AF.Tanh)
            nc.scalar.activation(ts["osb"][sl, :], gps[O_SL][gsl, :], AF.Sigmoid)
            nc.vector.tensor_mul(ts["hsb"][sl, :], ts["osb"][sl, :], ts["tcsb"][sl, :])

        def emit_transpose(h_src, ident_blk, state_tag):
            """h_src [64, 512] -> new [128, KC*B] transposed state tile."""
            tp = tpool.tile([128, KC * B], F32, tag="tp", name="tp" + state_tag)
            for k in range(KC):
                nc.tensor.transpose(tp[:, k * B:(k + 1) * B],
                                    h_src[:, k * 128:(k + 1) * 128], ident_blk)
            new = states.tile([128, KC * B], MDT, tag=state_tag, name=state_tag)
            nc.vector.tensor_copy(new, tp)
            return new

        def emit_pair(s, top, bottom):
            """TOP: layer-1 cell @ time s.  BOTTOM: layer-0 cell @ time s+1."""
            nonlocal h0T, h1T, c_prev, c1_prev, c0_prev, dec_w
            u = s + 1  # bottom timestep
            if bottom and u >= n_enc and dec_w is None:
                dec_w = load_wset(1)
            wtop = enc_w if (top and s < n_enc) else dec_w
            wbot = enc_w if (bottom and u < n_enc) else dec_w
            feedback = bottom and u > n_enc  # bottom x comes from this pair's head

            mm_h0T, mm_h1T = h0T, h1T
            stage = None
            if bottom:
                if not feedback:  # encoder steps + first decoder step from DRAM
                    stage = xpool.tile([2, B], MDT, tag="xstage")
                    nc.sync.dma_start(out=stage, in_=xaug[u, :, :])
                else:
                    stage = dec_stage

            # ---------------- matmul passes ----------------
            if PAIRED:
                gps_t = gps_b = [gpool.tile([128, 512], F32, tag="gp", name=f"gp{j}")
                                 for j in range(NCH)]
                bot_gsl, bot_tpos = BOT, (0, 64)
            else:
                gps_t = [gpool.tile([64, 512], F32, tag="gp", name=f"gpt{j}")
                         for j in range(NCH)] if top else None
                gps_b = [gpool.tile([64, 512], F32, tag="gp", name=f"gpb{j}")
                         for j in range(NCH)] if bottom else None
                bot_gsl, bot_tpos = slice(0, 64), (0, 0)

            a_seq = []  # top: bias1, wih1 x16, whh1 x16
            b_seq = []  # bottom: whh0 x16 (+ xb x4 if not feedback)
            first_b = [True] * NCH
            if top:
                for j in range(NCH):
                    a_seq.append((gps_t[j][TOP, :], ones64[0:1, :],
                                  wtop["r1"][0:1, j * 512:(j + 1) * 512], True, False))
                for j in range(NCH):
                    for k in range(KC):
                        a_seq.append((gps_t[j][TOP, :], mm_h0T[:, k * B:(k + 1) * B],
                                      wtop["w1i"][:, k, j * 512:(j + 1) * 512], False, False))
                # chunk-contiguous: chunk j's gates complete in order i,f,g,o so
                # the ACT/cell chain starts while later chunks still accumulate
                for j in range(NCH):
                    for k in range(KC):
                        a_seq.append((gps_t[j][TOP, :], mm_h1T[:, k * B:(k + 1) * B],
                                      wtop["w1h"][:, k, j * 512:(j + 1) * 512],
                                      False, k == KC - 1))
            if bottom:
                for j in range(NCH):
                    if mm_h0T is not None:
                        for k in range(KC):
                            b_seq.append((gps_b[j][bot_gsl, :], mm_h0T[:, k * B:(k + 1) * B],
                                          wbot["w0"][:, k, j * 512:(j + 1) * 512],
                                          first_b[j], False))
                            first_b[j] = False
                    if not feedback:
                        b_seq.append((gps_b[j][bot_gsl, :], stage[0:2, :],
                                      wbot["r0"][0:2, j * 512:(j + 1) * 512],
                                      first_b[j], True))
                        first_b[j] = False

            # emission order: bias passes, then bottom-dense 1:1 with top, then
            # the rest of top.  The bottom cell's matmuls finish mid-pair so its
            # h.T (needed by almost all of the next pair) is ready by pair end.
            # emit_mms(phase=0) emits through the end of the bottom stream (the
            # caller then emits the bottom cell + transposes so they land
            # mid-stream in the PE queue); emit_mms(phase=1) emits the rest.
            nbias = NCH if top else 0
            na, nb = len(a_seq), len(b_seq)
            order = [("a", x) for x in a_seq[:nbias]]
            ia, ib = nbias, 0
            if PAIRED:
                # 1:1 zip: both column groups advance at their own full rate
                # (starts are pc-monotone but execution is concurrent), so the
                # bottom group finishes at ~nb passes while A streams on
                while ib < nb:
                    order.append(("b", b_seq[ib])); ib += 1
                    if ia < na:
                        order.append(("a", a_seq[ia])); ia += 1
            else:
                order.extend(("b", x) for x in b_seq)  # bottom block first, dense
                ib = nb
            split0 = len(order)
            order.extend(("a", x) for x in a_seq[ia:])
            # phase boundaries: [0: bias+bottom][1: ~12 top passes][2: rest]
            split1 = min(split0 + 12, len(order))

            def emit_mms(phase):
                lo, hi = [(0, split0), (split0, split1), (split1, len(order))][phase]
                for grp, (out, lhsT, rhs, st, sp) in order[lo:hi]:
                    nc.tensor.matmul(out, lhsT, rhs, start=st, stop=sp,
                                     tile_position=(0, 0) if grp == "a" else bot_tpos,
                                     skip_group_check=True)

            # ---------------- activations + cell + transpose ----------------
            if PAIRED:
                ts_t = ts_b = alloc_tset("")
                c_new = states.tile([128, 512], F32, tag="c", name="c")
                if not (top and bottom):
                    nc.vector.memset(c_new[BOT if top else TOP, :], 0.0)
                cell_top = lambda: emit_cell(gps_t, TOP, ts_t, TOP,
                                             c_prev[TOP, :], c_new[TOP, :])
                cell_bot = lambda: emit_cell(gps_b, BOT, ts_b, BOT,
                                             c_prev[BOT, :], c_new[BOT, :])
                top_h = lambda: ts_t["hsb"][TOP, :]
                bot_h = lambda: ts_b["hsb"][BOT, :]
                bot_ident = ident[64:128, 64:128]
            else:
                ts_t = alloc_tset("t") if top else None
                ts_b = alloc_tset("b") if bottom else None
                c1_new = (states.tile([64, 512], F32, tag="c1", name="c1")
                          if top else None)
                c0_new = (states.tile([64, 512], F32, tag="c0", name="c0")
                          if bottom else None)
                cell_top = lambda: emit_cell(gps_t, slice(0, 64), ts_t, TOP,
                                             c1_prev[:, :], c1_new[:, :])
                cell_bot = lambda: emit_cell(gps_b, slice(0, 64), ts_b, TOP,
                                             c0_prev[:, :], c0_new[:, :])
                top_h = lambda: ts_t["hsb"][TOP, :]
                bot_h = lambda: ts_b["hsb"][TOP, :]
                bot_ident = ident[0:64, 0:64]

            def head():
                d = s - n_enc
                hd = tpool.tile([128, KC * B], F32, tag="tp", name="hd")[0:1, 0:B]
                nc.tensor.matmul(hd, ones64[0:1, 0:1], headb[0:1, :],
                                 start=True, stop=False)
                for k in range(KC):
                    nc.tensor.matmul(hd, headt[:, k:k + 1], h1T[:, k * B:(k + 1) * B],
                                     start=False, stop=k == KC - 1)
                if d + 1 < n_dec:
                    nc.vector.tensor_copy(dec_stage[0:1, :], hd)
                step_sb = acts.tile([1, B], F32, tag="stepsb", name="stepsb")
                nc.vector.tensor_copy(step_sb, hd)
                nc.sync.dma_start(out=outT[d:d + 1, :], in_=step_sb)

            if feedback:
                # decoder: top cell -> head -> bottom x pass -> bottom cell
                emit_mms(0)
                emit_mms(1)
                emit_mms(2)
                cell_top()
                h1T = emit_transpose(top_h(), ident[0:64, 0:64], "h1T")
                head()
                for j in range(NCH):
                    nc.tensor.matmul(gps_b[j][bot_gsl, :], stage[0:2, :],
                                     wbot["r0"][0:2, j * 512:(j + 1) * 512],
                                     start=first_b[j], stop=True,
                                     tile_position=bot_tpos, skip_group_check=True)
                cell_bot()
                h0T = emit_transpose(bot_h(), bot_ident, "h0T")
            else:
                # bottom first; its transposes go into the PE stream a dozen
                # passes later so the PE reaches them just as the bottom
                # cell's ACT/DVE chain finishes (no head-of-line stall)
                emit_mms(0)
                if bottom:
                    cell_bot()
                emit_mms(1)
                if bottom:
                    h0T = emit_transpose(bot_h(), bot_ident, "h0T")
                emit_mms(2)
                if top:
                    cell_top()
                    h1T = emit_transpose(top_h(), ident[0:64, 0:64], "h1T")
                    if s >= n_enc:
                        head()

            if PAIRED:
                c_prev = c_new
            else:
                if top:
                    c1_prev = c1_new
                if bottom:
                    c0_prev = c0_new

        # prologue: layer-0 @ t=0 alone
        emit_pair(-1, top=False, bottom=True)
        for s in range(NT - 1):
            emit_pair(s, top=True, bottom=True)
        emit_pair(NT - 1, top=True, bottom=False)

    nc.compile()
    return nc


# ------------------------------------------------------------------
# host-side packing
# ------------------------------------------------------------------
def _np_dt(mm_dt):
    if mm_dt == "bfloat16":
        import ml_dtypes
        return ml_dtypes.bfloat16
    return np.float32


def _pack_weights(inputs, mm_dt="float32r"):
    f32 = np.float32
    ndt = _np_dt(mm_dt)

    def wt_pack(w):  # [G, H] -> [128, KC, G]
        return np.ascontiguousarray(
            np.asarray(w, f32).T.reshape(KC, 128, G).transpose(1, 0, 2)).astype(ndt)

    m = {
        "wt_e0": wt_pack(inputs["enc_Whh0"]),
        "wt_e1i": wt_pack(inputs["enc_Wih1"]),
        "wt_e1h": wt_pack(inputs["enc_Whh1"]),
        "wt_d0": wt_pack(inputs["dec_Whh0"]),
        "wt_d1i": wt_pack(inputs["dec_Wih1"]),
        "wt_d1h": wt_pack(inputs["dec_Whh1"]),
        "rows_e0": np.stack([np.asarray(inputs["enc_Wih0"], f32)[:, 0],
                             np.asarray(inputs["enc_b0"], f32)]).astype(ndt),
        "rows_e1": np.asarray(inputs["enc_b1"], f32)[None, :].astype(ndt),
        "rows_d0": np.stack([np.asarray(inputs["dec_Wih0"], f32)[:, 0],
                             np.asarray(inputs["dec_b0"], f32)]).astype(ndt),
        "rows_d1": np.asarray(inputs["dec_b1"], f32)[None, :].astype(ndt),
        "headt": np.ascontiguousarray(
            np.asarray(inputs["head_W"], f32)[0].reshape(KC, 128).T).astype(ndt),
        "headb": np.full((1, B), float(np.asarray(inputs["head_b"])[0]), ndt),
        "zeros": np.zeros((128, KC * B), ndt),
    }
    return {k: np.ascontiguousarray(v) for k, v in m.items()}


def _pack_x(xc, n_enc=T, mm_dt="float32r"):
    """xc [B, T, 1] slice -> xaug [n_enc+1, 2, B]."""
    f32 = np.float32
    xt = np.asarray(xc, f32)[:, :, 0].T  # [T, B]
    xa = np.empty((n_enc + 1, 2, B), f32)
    xa[:n_enc, 0, :] = xt[:n_enc]
    xa[n_enc, 0, :] = xt[T - 1]  # decoder initial input = last observed x
    xa[:, 1, :] = 1.0
    return np.ascontiguousarray(xa.astype(_np_dt(mm_dt)))


def kernel(**inputs):
    global LAST_EXEC_NS, LAST_RESULTS
    from concourse.bass_utils import run_bass_kernel_spmd

    n_enc = int(os.environ.get("LSTM_NENC", T))
    n_dec = int(os.environ.get("LSTM_NDEC", HORIZON))
    mm_dt = os.environ.get("LSTM_MMDT", "float32r")
    key = (n_enc, n_dec, mm_dt)
    if key not in _CACHE:
        _CACHE[key] = _build(n_enc, n_dec, mm_dt)
    nc = _CACHE[key]

    shared = _pack_weights(inputs, mm_dt)
    in_maps = []
    for c in range(NCORES):
        m = dict(shared)
        m["xaug"] = _pack_x(inputs["x"][c * B:(c + 1) * B], n_enc, mm_dt)
        in_maps.append(m)

    trace = os.environ.get("LSTM_TRACE", "0") == "1"
    t0 = time.time()
    res = run_bass_kernel_spmd(nc, in_maps, list(range(NCORES)), trace=trace)
    wall = time.time() - t0
    LAST_EXEC_NS = res.exec_time_ns
    LAST_RESULTS = res
    if os.environ.get("LSTM_VERBOSE", "0") == "1":
        print(f"[kernel] wall={wall:.2f}s exec_time_ns={res.exec_time_ns}", file=sys.stderr)

    out = np.empty((B_FULL, n_dec), np.float32)
    for c in range(NCORES):
        out[c * B:(c + 1) * B, :] = res.results[c]["outT"].T
    return out


if __name__ == "__main__":
    cmd = sys.argv[1] if len(sys.argv) > 1 else "build"
    if cmd == "build":
        ne = int(os.environ.get("LSTM_NENC", "4"))
        nd = int(os.environ.get("LSTM_NDEC", "2"))
        md = os.environ.get("LSTM_MMDT", "float32r")
        t0 = time.time()
        nc = _build(ne, nd, md)
        print(f"build({ne},{nd},{md}) ok in {time.time()-t0:.1f}s")



# revision 12
# speedup vs baseline: 1.2287x; 1.2287x over previous
"""Trainium2 Bass kernel for a 2-layer encoder-decoder LSTM.

Problem: x [512, 256, 1] -> encoder 2-layer LSTM (H=512) -> autoregressive
decoder (64 steps, head feedback) -> out [512, 64].

Strategy: data-parallel across 8 NeuronCores (batch 512 -> 64 per core), all
weights replicated and SBUF-resident.  Per core each timestep needs 3
matmuls of [64,512] @ [512,2048], run "activation-stationary" (lhsT = h.T
chunk [128,64], moving = W.T [128,512] slices).  Each "pair" step computes
layer-1 @ t together with layer-0 @ t+1 so the PE always has a deep stream
of independent work.

Two dtype modes (LSTM_MMDT):
 - bfloat16: PE column tiling packs the two cells onto separate column
   groups (layer-1 -> PSUM partitions 0..63, layer-0 -> 64..127) which run
   concurrently, and the activation/vector engines process both cells
   stacked [128, *] at full width.
 - float32r: tf32-like full-width mode (the PE uses both columns of each
   column pair, so no column tiling is possible); the two cells use
   separate PSUM tiles on partitions 0..63.

Biases and the scalar input term are folded into extra K=1/K=2 matmul
accumulation passes.  h is re-transposed each step with PE transpose; the
transposed h.T doubles as the moving operand of the decoder head matmul.
"""

import os
import sys
import time

import numpy as np

B_FULL, T, H, HORIZON = 512, 256, 512, 64
NCORES = 8
B = B_FULL // NCORES          # 64 batch rows per core
G = 4 * H                     # 2048 gate columns
KC = H // 128                 # 4 contraction chunks
NCH = G // 512                # 4 output chunks of 512 (one PSUM bank each)

# gate chunk indices (PyTorch order: i, f, g, o)
I_SL, F_SL, G_SL, O_SL = 0, 1, 2, 3

_CACHE = {}
LAST_EXEC_NS = None
LAST_RESULTS = None


def _build(n_enc=T, n_dec=HORIZON, mm_dt="float32r"):
    """Build the Bass module (single SPMD program, run on all 8 cores)."""
    from contextlib import ExitStack

    import concourse.mybir as mybir
    import concourse.tile as tile
    from concourse import bacc
    from concourse.masks import make_identity

    dt = mybir.dt
    MDT = getattr(dt, mm_dt)
    F32 = dt.float32
    AF = mybir.ActivationFunctionType
    NT = n_enc + n_dec            # total timesteps
    PAIRED = mm_dt != "float32r"  # col-tiled two-group mode

    nc = bacc.Bacc("TRN2", target_bir_lowering=False, debug=False)

    # ---------------- DRAM parameters (per-core views) ----------------
    xaug = nc.declare_dram_parameter("xaug", [n_enc + 1, 2, B], MDT, isOutput=False)
    wt = {}
    for nm in ("e0", "e1i", "e1h", "d0", "d1i", "d1h"):
        wt[nm] = nc.declare_dram_parameter(f"wt_{nm}", [128, KC, G], MDT, isOutput=False)
    rows_e0 = nc.declare_dram_parameter("rows_e0", [2, G], MDT, isOutput=False)
    rows_e1 = nc.declare_dram_parameter("rows_e1", [1, G], MDT, isOutput=False)
    rows_d0 = nc.declare_dram_parameter("rows_d0", [2, G], MDT, isOutput=False)
    rows_d1 = nc.declare_dram_parameter("rows_d1", [1, G], MDT, isOutput=False)
    headt_d = nc.declare_dram_parameter("headt", [128, KC], MDT, isOutput=False)
    headb_d = nc.declare_dram_parameter("headb", [1, B], MDT, isOutput=False)
    zeros_d = nc.declare_dram_parameter("zeros", [128, KC * B], MDT, isOutput=False)
    outT = nc.declare_dram_parameter("outT", [n_dec, B], F32, isOutput=True)

    with ExitStack() as ctx:
        tc = ctx.enter_context(tile.TileContext(nc))
        wpool = ctx.enter_context(tc.tile_pool(name="w", bufs=1))
        consts = ctx.enter_context(tc.tile_pool(name="consts", bufs=1))
        states = ctx.enter_context(tc.tile_pool(name="states", bufs=2))
        acts = ctx.enter_context(tc.tile_pool(name="acts", bufs=2))
        xpool = ctx.enter_context(tc.tile_pool(name="xp", bufs=4))
        gpool = ctx.enter_context(
            tc.tile_pool(name="gp", bufs=6, space="PSUM"))
        tpool = ctx.enter_context(tc.tile_pool(name="tp", bufs=1, space="PSUM"))

        # ---------------- constants ----------------
        ident = consts.tile([128, 128], F32, tag="ident")
        make_identity(nc, ident)
        ones64 = consts.tile([1, B], MDT, tag="ones64")
        nc.sync.dma_start(out=ones64, in_=xaug[0, 1:2, :])
        headt = consts.tile([128, KC], MDT, tag="headt")
        nc.sync.dma_start(out=headt, in_=headt_d[:, :])
        headb = consts.tile([1, B], MDT, tag="headb")
        nc.sync.dma_start(out=headb, in_=headb_d[:, :])
        dec_stage = consts.tile([2, B], MDT, tag="dec_stage")
        # (x_last, ones): row 1 stays 1.0 forever; row 0 overwritten per step
        nc.sync.dma_start(out=dec_stage, in_=xaug[n_enc, :, :])

        # weight tiles: encoder set now, decoder set later (same tags)
        def load_wset(phase):
            p = "e" if phase == 0 else "d"
            w0 = wpool.tile([128, KC, G], MDT, tag="w0")
            nc.sync.dma_start(out=w0, in_=wt[p + "0"][:, :, :])
            w1i = wpool.tile([128, KC, G], MDT, tag="w1i")
            nc.sync.dma_start(out=w1i, in_=wt[p + "1i"][:, :, :])
            w1h = wpool.tile([128, KC, G], MDT, tag="w1h")
            nc.sync.dma_start(out=w1h, in_=wt[p + "1h"][:, :, :])
            r0 = wpool.tile([2, G], MDT, tag="rows0")
            nc.sync.dma_start(out=r0, in_=(rows_e0 if phase == 0 else rows_d0)[:, :])
            r1 = wpool.tile([1, G], MDT, tag="rows1")
            nc.sync.dma_start(out=r1, in_=(rows_e1 if phase == 0 else rows_d1)[:, :])
            return dict(w0=w0, w1i=w1i, w1h=w1h, r0=r0, r1=r1)

        enc_w = load_wset(0)
        dec_w = None

        TOP = slice(0, 64)
        BOT = slice(64, 128)

        # ---------------- initial state ----------------
        h0T = None                       # [128, KC*B] transposed h0 (set by prologue)
        h1T = states.tile([128, KC * B], MDT, tag="h1T")
        nc.sync.dma_start(out=h1T, in_=zeros_d[:, :])
        if PAIRED:
            c_prev = states.tile([128, 512], F32, tag="c")
            nc.vector.memset(c_prev, 0.0)
            c1_prev = c0_prev = None
        else:
            c1_prev = states.tile([64, 512], F32, tag="c1")
            nc.vector.memset(c1_prev, 0.0)
            c0_prev = states.tile([64, 512], F32, tag="c0")
            nc.vector.memset(c0_prev, 0.0)
            c_prev = None

        def alloc_tset(sfx):
            return dict(
                ifsb=acts.tile([128, 1024], F32, tag="ifsb" + sfx, name="ifsb" + sfx),
                gsb=acts.tile([128, 512], F32, tag="gsb" + sfx, name="gsb" + sfx),
                osb=acts.tile([128, 512], F32, tag="osb" + sfx, name="osb" + sfx),
                t1=acts.tile([128, 512], F32, tag="t1" + sfx, name="t1" + sfx),
                t2=acts.tile([128, 512], F32, tag="t2" + sfx, name="t2" + sfx),
                tcsb=acts.tile([128, 512], F32, tag="tcsb" + sfx, name="tcsb" + sfx),
                hsb=acts.tile([128, 512], F32, tag="hsb" + sfx, name="hsb" + sfx),
            )

        def emit_cell(gps, gsl, ts, sl, c_prev_ap, c_new_ap):
            """One LSTM cell's activations + state update.
            gps: 4 psum chunk tiles; gsl: partition slice in psum;
            ts: act tile set; sl: partition slice in act tiles."""
            nc.scalar.activation(ts["ifsb"][sl, 0:512], gps[I_SL][gsl, :], AF.Sigmoid)
            nc.scalar.activation(ts["ifsb"][sl, 512:1024], gps[F_SL][gsl, :], AF.Sigmoid)
            nc.scalar.activation(ts["gsb"][sl, :], gps[G_SL][gsl, :], AF.Tanh)
            nc.vector.tensor_mul(ts["t1"][sl, :], ts["ifsb"][sl, 512:1024], c_prev_ap)
            nc.vector.tensor_mul(ts["t2"][sl, :], ts["ifsb"][sl, 0:512], ts["gsb"][sl, :])
            nc.vector.tensor_add(c_new_ap, ts["t1"][sl, :], ts["t2"][sl, :])
            # tanh(c) before sigmoid(o) in ACT program order: o depends on the
            # last-finishing gate chunk, tanh(c) only on i/f/g
            nc.scalar.activation(ts["tcsb"][sl, :], c_new_ap, AF.Tanh)
            nc.scalar.activation(ts["osb"][sl, :], gps[O_SL][gsl, :], AF.Sigmoid)
            nc.vector.tensor_mul(ts["hsb"][sl, :], ts["osb"][sl, :], ts["tcsb"][sl, :])

        def emit_transpose(h_src, ident_blk, state_tag):
            """h_src [64, 512] -> new [128, KC*B] transposed state tile."""
            tp = tpool.tile([128, KC * B], F32, tag="tp", name="tp" + state_tag)
            for k in range(KC):
                nc.tensor.transpose(tp[:, k * B:(k + 1) * B],
                                    h_src[:, k * 128:(k + 1) * 128], ident_blk)
            new = states.tile([128, KC * B], MDT, tag=state_tag, name=state_tag)
            nc.vector.tensor_copy(new, tp)
            return new

        def emit_pair(s, top, bottom):
            """TOP: layer-1 cell @ time s.  BOTTOM: layer-0 cell @ time s+1."""
            nonlocal h0T, h1T, c_prev, c1_prev, c0_prev, dec_w
            u = s + 1  # bottom timestep
            if bottom and u >= n_enc and dec_w is None:
                dec_w = load_wset(1)
            wtop = enc_w if (top and s < n_enc) else dec_w
            wbot = enc_w if (bottom and u < n_enc) else dec_w
            feedback = bottom and u > n_enc  # bottom x comes from this pair's head

            mm_h0T, mm_h1T = h0T, h1T
            stage = None
            if bottom:
                if not feedback:  # encoder steps + first decoder step from DRAM
                    stage = xpool.tile([2, B], MDT, tag="xstage")
                    nc.sync.dma_start(out=stage, in_=xaug[u, :, :])
                else:
                    stage = dec_stage

            # ---------------- matmul passes ----------------
            if PAIRED:
                gps_t = gps_b = [gpool.tile([128, 512], F32, tag="gp", name=f"gp{j}")
                                 for j in range(NCH)]
                bot_gsl, bot_tpos = BOT, (0, 64)
            else:
                gps_t = [gpool.tile([64, 512], F32, tag="gp", name=f"gpt{j}")
                         for j in range(NCH)] if top else None
                gps_b = [gpool.tile([64, 512], F32, tag="gp", name=f"gpb{j}")
                         for j in range(NCH)] if bottom else None
                bot_gsl, bot_tpos = slice(0, 64), (0, 0)

            a_seq = []  # top: bias1, wih1 x16, whh1 x16
            b_seq = []  # bottom: whh0 x16 (+ xb x4 if not feedback)
            first_b = [True] * NCH
            if top:
                for j in range(NCH):
                    a_seq.append((gps_t[j][TOP, :], ones64[0:1, :],
                                  wtop["r1"][0:1, j * 512:(j + 1) * 512], True, False))
                for j in range(NCH):
                    for k in range(KC):
                        a_seq.append((gps_t[j][TOP, :], mm_h0T[:, k * B:(k + 1) * B],
                                      wtop["w1i"][:, k, j * 512:(j + 1) * 512], False, False))
                # chunk-contiguous: chunk j's gates complete in order i,f,g,o so
                # the ACT/cell chain starts while later chunks still accumulate
                for j in range(NCH):
                    for k in range(KC):
                        a_seq.append((gps_t[j][TOP, :], mm_h1T[:, k * B:(k + 1) * B],
                                      wtop["w1h"][:, k, j * 512:(j + 1) * 512],
                                      False, k == KC - 1))
            if bottom:
                for j in range(NCH):
                    if mm_h0T is not None:
                        for k in range(KC):
                            b_seq.append((gps_b[j][bot_gsl, :], mm_h0T[:, k * B:(k + 1) * B],
                                          wbot["w0"][:, k, j * 512:(j + 1) * 512],
                                          first_b[j], False))
                            first_b[j] = False
                    if not feedback:
                        b_seq.append((gps_b[j][bot_gsl, :], stage[0:2, :],
                                      wbot["r0"][0:2, j * 512:(j + 1) * 512],
                                      first_b[j], True))
                        first_b[j] = False

            # emission order: bias passes, then bottom-dense 1:1 with top, then
            # the rest of top.  The bottom cell's matmuls finish mid-pair so its
            # h.T (needed by almost all of the next pair) is ready by pair end.
            # emit_mms(phase=0) emits through the end of the bottom stream (the
            # caller then emits the bottom cell + transposes so they land
            # mid-stream in the PE queue); emit_mms(phase=1) emits the rest.
            nbias = NCH if top else 0
            na, nb = len(a_seq), len(b_seq)
            order = [("a", x) for x in a_seq[:nbias]]
            ia, ib = nbias, 0
            if PAIRED:
                # 1:1 zip: both column groups advance at their own full rate
                # (starts are pc-monotone but execution is concurrent), so the
                # bottom group finishes at ~nb passes while A streams on
                while ib < nb:
                    order.append(("b", b_seq[ib])); ib += 1
                    if ia < na:
                        order.append(("a", a_seq[ia])); ia += 1
            else:
                order.extend(("b", x) for x in b_seq)  # bottom block first, dense
                ib = nb
            split0 = len(order)
            order.extend(("a", x) for x in a_seq[ia:])
            # phase boundaries: [0: bias+bottom][1: ~12 top passes][2: rest]
            split1 = min(split0 + 12, len(order))

            def emit_mms(phase):
                lo, hi = [(0, split0), (split0, split1), (split1, len(order))][phase]
                for grp, (out, lhsT, rhs, st, sp) in order[lo:hi]:
                    nc.tensor.matmul(out, lhsT, rhs, start=st, stop=sp,
                                     tile_position=(0, 0) if grp == "a" else bot_tpos,
                                     skip_group_check=True)

            # ---------------- activations + cell + transpose ----------------
            if PAIRED:
                ts_t = ts_b = alloc_tset("")
                c_new = states.tile([128, 512], F32, tag="c", name="c")
                if not (top and bottom):
                    nc.vector.memset(c_new[BOT if top else TOP, :], 0.0)
                cell_top = lambda: emit_cell(gps_t, TOP, ts_t, TOP,
                                             c_prev[TOP, :], c_new[TOP, :])
                cell_bot = lambda: emit_cell(gps_b, BOT, ts_b, BOT,
                                             c_prev[BOT, :], c_new[BOT, :])
                top_h = lambda: ts_t["hsb"][TOP, :]
                bot_h = lambda: ts_b["hsb"][BOT, :]
                bot_ident = ident[64:128, 64:128]
            else:
                ts_t = alloc_tset("t") if top else None
                ts_b = alloc_tset("b") if bottom else None
                c1_new = (states.tile([64, 512], F32, tag="c1", name="c1")
                          if top else None)
                c0_new = (states.tile([64, 512], F32, tag="c0", name="c0")
                          if bottom else None)
                cell_top = lambda: emit_cell(gps_t, slice(0, 64), ts_t, TOP,
                                             c1_prev[:, :], c1_new[:, :])
                cell_bot = lambda: emit_cell(gps_b, slice(0, 64), ts_b, TOP,
                                             c0_prev[:, :], c0_new[:, :])
                top_h = lambda: ts_t["hsb"][TOP, :]
                bot_h = lambda: ts_b["hsb"][TOP, :]
                bot_ident = ident[0:64, 0:64]

            def head():
                d = s - n_enc
                hd = tpool.tile([128, KC * B], F32, tag="tp", name="hd")[0:1, 0:B]
                nc.tensor.matmul(hd, ones64[0:1, 0:1], headb[0:1, :],
                                 start=True, stop=False)
                for k in range(KC):
                    nc.tensor.matmul(hd, headt[:, k:k + 1], h1T[:, k * B:(k + 1) * B],
                                     start=False, stop=k == KC - 1)
                if d + 1 < n_dec:
                    nc.vector.tensor_copy(dec_stage[0:1, :], hd)
                step_sb = acts.tile([1, B], F32, tag="stepsb", name="stepsb")
                nc.vector.tensor_copy(step_sb, hd)
                nc.sync.dma_start(out=outT[d:d + 1, :], in_=step_sb)

            if feedback:
                # decoder: top cell -> head -> bottom x pass -> bottom cell
                emit_mms(0)
                emit_mms(1)
                emit_mms(2)
                cell_top()
                h1T = emit_transpose(top_h(), ident[0:64, 0:64], "h1T")
                head()
                for j in range(NCH):
                    nc.tensor.matmul(gps_b[j][bot_gsl, :], stage[0:2, :],
                                     wbot["r0"][0:2, j * 512:(j + 1) * 512],
                                     start=first_b[j], stop=True,
                                     tile_position=bot_tpos, skip_group_check=True)
                cell_bot()
                h0T = emit_transpose(bot_h(), bot_ident, "h0T")
            else:
                # bottom first; its transposes go into the PE stream a dozen
                # passes later so the PE reaches them just as the bottom
                # cell's ACT/DVE chain finishes (no head-of-line stall)
                emit_mms(0)
                if bottom:
                    cell_bot()
                emit_mms(1)
                if bottom:
                    h0T = emit_transpose(bot_h(), bot_ident, "h0T")
                emit_mms(2)
                if top:
                    cell_top()
                    h1T = emit_transpose(top_h(), ident[0:64, 0:64], "h1T")
                    if s >= n_enc:
                        head()

            if PAIRED:
                c_prev = c_new
            else:
                if top:
                    c1_prev = c1_new
                if bottom:
                    c0_prev = c0_new

        # prologue: layer-0 @ t=0 alone
        emit_pair(-1, top=False, bottom=True)
        for s in range(NT - 1):
            emit_pair(s, top=True, bottom=True)
        emit_pair(NT - 1, top=True, bottom=False)

    nc.compile()
    return nc


# ------------------------------------------------------------------
# host-side packing
# ------------------------------------------------------------------
def _np_dt(mm_dt):
    if mm_dt == "bfloat16":
        import ml_dtypes
        return ml_dtypes.bfloat16
    return np.float32


def _pack_weights(inputs, mm_dt="float32r"):
    f32 = np.float32
    ndt = _np_dt(mm_dt)

    def wt_pack(w):  # [G, H] -> [128, KC, G]
        return np.ascontiguousarray(
            np.asarray(w, f32).T.reshape(KC, 128, G).transpose(1, 0, 2)).astype(ndt)

    m = {
        "wt_e0": wt_pack(inputs["enc_Whh0"]),
        "wt_e1i": wt_pack(inputs["enc_Wih1"]),
        "wt_e1h": wt_pack(inputs["enc_Whh1"]),
        "wt_d0": wt_pack(inputs["dec_Whh0"]),
        "wt_d1i": wt_pack(inputs["dec_Wih1"]),
        "wt_d1h": wt_pack(inputs["dec_Whh1"]),
        "rows_e0": np.stack([np.asarray(inputs["enc_Wih0"], f32)[:, 0],
                             np.asarray(inputs["enc_b0"], f32)]).astype(ndt),
        "rows_e1": np.asarray(inputs["enc_b1"], f32)[None, :].astype(ndt),
        "rows_d0": np.stack([np.asarray(inputs["dec_Wih0"], f32)[:, 0],
                             np.asarray(inputs["dec_b0"], f32)]).astype(ndt),
        "rows_d1": np.asarray(inputs["dec_b1"], f32)[None, :].astype(ndt),
        "headt": np.ascontiguousarray(
            np.asarray(inputs["head_W"], f32)[0].reshape(KC, 128).T).astype(ndt),
        "headb": np.full((1, B), float(np.asarray(inputs["head_b"])[0]), ndt),
        "zeros": np.zeros((128, KC * B), ndt),
    }
    return {k: np.ascontiguousarray(v) for k, v in m.items()}


def _pack_x(xc, n_enc=T, mm_dt="float32r"):
    """xc [B, T, 1] slice -> xaug [n_enc+1, 2, B]."""
    f32 = np.float32
    xt = np.asarray(xc, f32)[:, :, 0].T  # [T, B]
    xa = np.empty((n_enc + 1, 2, B), f32)
    xa[:n_enc, 0, :] = xt[:n_enc]
    xa[n_enc, 0, :] = xt[T - 1]  # decoder initial input = last observed x
    xa[:, 1, :] = 1.0
    return np.ascontiguousarray(xa.astype(_np_dt(mm_dt)))


def kernel(**inputs):
    global LAST_EXEC_NS, LAST_RESULTS
    from concourse.bass_utils import run_bass_kernel_spmd

    n_enc = int(os.environ.get("LSTM_NENC", T))
    n_dec = int(os.environ.get("LSTM_NDEC", HORIZON))
    mm_dt = os.environ.get("LSTM_MMDT", "bfloat16")
    key = (n_enc, n_dec, mm_dt)
    if key not in _CACHE:
        if mm_dt == "bfloat16":
            _CACHE[key] = _build_v2(n_enc, n_dec)
        else:
            _CACHE[key] = _build(n_enc, n_dec, mm_dt)
    nc = _CACHE[key]

    shared = (_pack_weights_v2(inputs) if mm_dt == "bfloat16"
              else _pack_weights(inputs, mm_dt))
    in_maps = []
    for c in range(NCORES):
        m = dict(shared)
        m["xaug"] = _pack_x(inputs["x"][c * B:(c + 1) * B], n_enc, mm_dt)
        in_maps.append(m)

    trace = os.environ.get("LSTM_TRACE", "0") == "1"
    t0 = time.time()
    res = run_bass_kernel_spmd(nc, in_maps, list(range(NCORES)), trace=trace)
    wall = time.time() - t0
    LAST_EXEC_NS = res.exec_time_ns
    LAST_RESULTS = res
    if os.environ.get("LSTM_VERBOSE", "0") == "1":
        print(f"[kernel] wall={wall:.2f}s exec_time_ns={res.exec_time_ns}", file=sys.stderr)

    out = np.empty((B_FULL, n_dec), np.float32)
    for c in range(NCORES):
        r = res.results[c]["outT"]
        out[c * B:(c + 1) * B, :] = r if mm_dt == "bfloat16" else r.T
    return out


if __name__ == "__main__":
    cmd = sys.argv[1] if len(sys.argv) > 1 else "build"
    if cmd == "build":
        ne = int(os.environ.get("LSTM_NENC", "4"))
        nd = int(os.environ.get("LSTM_NDEC", "2"))
        md = os.environ.get("LSTM_MMDT", "float32r")
        t0 = time.time()
        nc = _build(ne, nd, md)
        print(f"build({ne},{nd},{md}) ok in {time.time()-t0:.1f}s")


# ------------------------------------------------------------------
# v2: software-pipelined PAIRED bf16 build
# ------------------------------------------------------------------
def _build_v2(n_enc=T, n_dec=HORIZON):
    """bf16 col-tiled build with a software-pipelined PE stream.

    Per iteration t (cell1@t TOP / cell0@t+1 BOT):
      [bias j=0..2] [T_top(t-1)] [head(t-1)] [bias j=3 (+fb BOT bias)]
      [w1h x16, T_bot(t) injected after 8] [zip w0(t+1)/w1i(t) x32] [x(t+1) x4]
    Decoder feedback is a DVE rank-1 update (x * Wih0_col) instead of a
    serial head->stage->matmul chain, so every iteration streams densely.
    """
    from contextlib import ExitStack

    import concourse.mybir as mybir
    import concourse.tile as tile
    from concourse import bacc
    from concourse.masks import make_identity

    dt = mybir.dt
    MDT = dt.bfloat16
    F32 = dt.float32
    AF = mybir.ActivationFunctionType
    NT = n_enc + n_dec

    nc = bacc.Bacc("TRN2", target_bir_lowering=False, debug=False)

    xaug = nc.declare_dram_parameter("xaug", [n_enc + 1, 2, B], MDT, isOutput=False)
    wt = {}
    for nm in ("e0", "e1i", "e1h", "d0", "d1i", "d1h"):
        wt[nm] = nc.declare_dram_parameter(f"wt_{nm}", [128, KC, G], MDT, isOutput=False)
    rows_e0 = nc.declare_dram_parameter("rows_e0", [2, G], MDT, isOutput=False)
    rows_e1 = nc.declare_dram_parameter("rows_e1", [1, G], MDT, isOutput=False)
    rows_d0 = nc.declare_dram_parameter("rows_d0", [2, G], MDT, isOutput=False)
    rows_d1 = nc.declare_dram_parameter("rows_d1", [1, G], MDT, isOutput=False)
    rows_fb = nc.declare_dram_parameter("rows_fb", [1, G], MDT, isOutput=False)
    wbx_d = nc.declare_dram_parameter("wbx", [B, G], MDT, isOutput=False)
    headt_d = nc.declare_dram_parameter("headt", [128, KC], MDT, isOutput=False)
    headb_d = nc.declare_dram_parameter("headb", [1, B], MDT, isOutput=False)
    zeros_d = nc.declare_dram_parameter("zeros", [128, KC * B], MDT, isOutput=False)
    outT = nc.declare_dram_parameter("outT", [B, n_dec], F32, isOutput=True)

    TOP = slice(0, 64)
    BOT = slice(64, 128)

    with ExitStack() as ctx:
        tc = ctx.enter_context(tile.TileContext(nc))
        wpool = ctx.enter_context(tc.tile_pool(name="w", bufs=1))
        consts = ctx.enter_context(tc.tile_pool(name="consts", bufs=1))
        states = ctx.enter_context(tc.tile_pool(name="states", bufs=3))
        acts = ctx.enter_context(tc.tile_pool(name="acts", bufs=2))
        fbpool = ctx.enter_context(tc.tile_pool(name="fb", bufs=1))
        xpool = ctx.enter_context(tc.tile_pool(name="xp", bufs=4))
        gpool = ctx.enter_context(tc.tile_pool(name="gp", bufs=6, space="PSUM"))
        tpool = ctx.enter_context(tc.tile_pool(name="tp", bufs=1, space="PSUM"))

        ident = consts.tile([128, 128], F32, tag="ident")
        make_identity(nc, ident)
        ones64 = consts.tile([1, B], MDT, tag="ones64")
        nc.sync.dma_start(out=ones64, in_=xaug[0, 1:2, :])
        headt = consts.tile([128, KC], MDT, tag="headt")
        nc.sync.dma_start(out=headt, in_=headt_d[:, :])
        headb = consts.tile([1, B], MDT, tag="headb")
        nc.sync.dma_start(out=headb, in_=headb_d[:, :])
        rfb = consts.tile([1, G], MDT, tag="rfb")
        nc.sync.dma_start(out=rfb, in_=rows_fb[:, :])
        wbx = consts.tile([B, G], MDT, tag="wbx")
        nc.sync.dma_start(out=wbx, in_=wbx_d[:, :])

        def load_wset(p, r0_d, r1_d):
            w0 = wpool.tile([128, KC, G], MDT, tag=f"w0{p}")
            nc.sync.dma_start(out=w0, in_=wt[p + "0"][:, :, :])
            w1i = wpool.tile([128, KC, G], MDT, tag=f"w1i{p}")
            nc.sync.dma_start(out=w1i, in_=wt[p + "1i"][:, :, :])
            w1h = wpool.tile([128, KC, G], MDT, tag=f"w1h{p}")
            nc.sync.dma_start(out=w1h, in_=wt[p + "1h"][:, :, :])
            r0 = wpool.tile([2, G], MDT, tag=f"r0{p}")
            nc.sync.dma_start(out=r0, in_=r0_d[:, :])
            r1 = wpool.tile([1, G], MDT, tag=f"r1{p}")
            nc.sync.dma_start(out=r1, in_=r1_d[:, :])
            return dict(w0=w0, w1i=w1i, w1h=w1h, r0=r0, r1=r1)

        enc_w = load_wset("e", rows_e0, rows_e1)
        dec_w = load_wset("d", rows_d0, rows_d1)

        h1T = states.tile([128, KC * B], MDT, tag="h1T", name="h1T_init")
        nc.sync.dma_start(out=h1T, in_=zeros_d[:, :])
        c_init = states.tile([128, 512], F32, tag="c", name="c_init")
        nc.vector.memset(c_init, 0.0)

        def alloc_tset(nm):
            return dict(
                ifsb=acts.tile([128, 1024], F32, tag="ifsb", name="ifsb" + nm),
                gsb=acts.tile([128, 512], F32, tag="gsb", name="gsb" + nm),
                osb=acts.tile([128, 512], F32, tag="osb", name="osb" + nm),
                t1=acts.tile([128, 512], F32, tag="t1", name="t1" + nm),
                t2=acts.tile([128, 512], F32, tag="t2", name="t2" + nm),
                tcsb=acts.tile([128, 512], F32, tag="tcsb", name="tcsb" + nm),
                hsb=acts.tile([128, 512], F32, tag="hsb", name="hsb" + nm),
            )

        def emit_cell(gaps, ts, sl, cp, cn):
            # i/f/g full width; the o-tail (tanh_c, sig_o, h-mul) in halves so
            # the first transposes can start ~600ns earlier
            nc.scalar.activation(ts["ifsb"][sl, 0:512], gaps[0], AF.Sigmoid)
            nc.scalar.activation(ts["ifsb"][sl, 512:1024], gaps[1], AF.Sigmoid)
            nc.scalar.activation(ts["gsb"][sl, :], gaps[2], AF.Tanh)
            nc.vector.tensor_mul(ts["t1"][sl, :], ts["ifsb"][sl, 512:1024], cp)
            nc.vector.tensor_mul(ts["t2"][sl, :], ts["ifsb"][sl, 0:512], ts["gsb"][sl, :])
            for h in (slice(0, 256), slice(256, 512)):
                nc.vector.tensor_add(cn[:, h] if cn.partition_size() == 128
                                     else cn[:, h], ts["t1"][sl, h], ts["t2"][sl, h])
            for h in (slice(0, 256), slice(256, 512)):
                nc.scalar.activation(ts["tcsb"][sl, h],
                                     (cn[:, h] if cn.partition_size() == 128
                                      else cn[:, h]), AF.Tanh)
                nc.scalar.activation(ts["osb"][sl, h], gaps[3][:, h], AF.Sigmoid)
                nc.vector.tensor_mul(ts["hsb"][sl, h], ts["osb"][sl, h],
                                     ts["tcsb"][sl, h])

        def transpose4_mm(tt, col0, h_src, ident_blk):
            for k in range(KC):
                nc.tensor.transpose(tt[:, col0 + k * B:col0 + (k + 1) * B],
                                    h_src[:, k * 128:(k + 1) * 128], ident_blk)

        def transpose4_cp(tt, col0, state_tag, nm):
            new = states.tile([128, KC * B], MDT, tag=state_tag, name=nm)
            nc.vector.tensor_copy(new[:, 0:128], tt[:, col0:col0 + 128])
            nc.vector.tensor_copy(new[:, 128:256], tt[:, col0 + 128:col0 + 256])
            return new

        def transpose4(tt, col0, h_src, ident_blk, state_tag, nm):
            transpose4_mm(tt, col0, h_src, ident_blk)
            return transpose4_cp(tt, col0, state_tag, nm)

        # ---------------- prologue: cell0@0 ----------------
        stages = {}

        def fetch_stage(u):
            if u <= n_enc:
                st = xpool.tile([2, B], MDT, tag="xstage", name=f"xs{u}")
                nc.sync.dma_start(out=st, in_=xaug[u, :, :])
                stages[u] = st

        fetch_stage(0)
        fetch_stage(1)
        gps_prev = [gpool.tile([128, 512], F32, tag="gp", name=f"gpp{j}")
                    for j in range(NCH)]
        for j in range(NCH):
            nc.tensor.matmul(gps_prev[j][BOT, :], stages[0][0:2, :],
                             enc_w["r0"][0:2, j * 512:(j + 1) * 512],
                             start=True, stop=True, tile_position=(0, 64),
                             skip_group_check=True)
        ts_prev = alloc_tset("pro")
        c_prev2 = c_init                      # {c1@t-2, c0@t-1} slot
        c_prev = states.tile([128, 512], F32, tag="c", name="c0")
        nc.vector.memset(c_prev[TOP, :], 0.0)  # c1@-1 = 0
        emit_cell([gps_prev[j][BOT, :] for j in range(NCH)], ts_prev, BOT,
                  c_init[BOT, :], c_prev[BOT, :])
        ttB = tpool.tile([128, 256], F32, tag="ttB", name="ttB_pro")
        h0T = transpose4(ttB, 0, ts_prev["hsb"][BOT, :], ident[64:128, 64:128],
                         "h0T", "h0T_0")

        # ---------------- main loop ----------------
        # iter t: TOP = cell1@t, BOT(F) = cell0@t+1 dense.
        # ts(t)/c(t) = {cell1@t, cell0@t+1}; chains stacked when u<=n_enc.
        for t in range(NT):
            u = t + 1
            wt_t = enc_w if t < n_enc else dec_w
            wt_b = enc_w if u < n_enc else dec_w
            fb_u = u > n_enc            # cell0@u head-fed (D at iter u)
            fb_t = t > n_enc            # cell0@t head-fed (D this iter)
            have_bot = u < NT
            enc_x = (not fb_u) and have_bot

            ttA = tpool.tile([128, 256], F32, tag="ttA", name=f"ttA{t}")
            ttB = tpool.tile([128, 256], F32, tag="ttB", name=f"ttB{t}")
            fetch_stage(u + 1)

            gps = [gpool.tile([128, 512], F32, tag="gp", name=f"gp{t}_{j}")
                   for j in range(NCH)]

            # front: x-pass(u) from prefetched stage (encoder-fed bottoms)
            if enc_x:
                stage_u = stages.pop(u)
                for j in range(NCH):
                    nc.tensor.matmul(gps[j][BOT, :], stage_u[0:2, :],
                                     wt_b["r0"][0:2, j * 512:(j + 1) * 512],
                                     start=True, stop=False,
                                     tile_position=(0, 64),
                                     skip_group_check=True)
            for j in range(3):
                nc.tensor.matmul(gps[j][TOP, :], ones64[0:1, :],
                                 wt_t["r1"][0:1, j * 512:(j + 1) * 512],
                                 start=True, stop=False, tile_position=(0, 0),
                                 skip_group_check=True)
            # T_top(t-1); T_bot(t) too when cell0@t came from the stacked chain
            if t >= 1:
                transpose4_mm(ttA, 0, ts_prev["hsb"][TOP, :], ident[0:64, 0:64])
                if not fb_t:
                    transpose4_mm(ttB, 0, ts_prev["hsb"][BOT, :],
                                  ident[64:128, 64:128])
                h1T = transpose4_cp(ttA, 0, "h1T", f"h1T_{t}")
                if not fb_t:
                    h0T = transpose4_cp(ttB, 0, "h0T", f"h0T_{t}")
            # head(t-1) -> output step (+ xcol for D)
            xcol = None
            if t - 1 >= n_enc:
                d = t - 1 - n_enc
                hd = ttB[0:64, 255:256]
                nc.tensor.matmul(hd, ones64[0:1, :], headb[0:1, 0:1],
                                 start=True, stop=False, skip_group_check=True)
                for k in range(KC):
                    nc.tensor.matmul(hd, h1T[:, k * B:(k + 1) * B],
                                     headt[:, k:k + 1], start=False,
                                     stop=k == KC - 1, skip_group_check=True)
                step_sb = acts.tile([64, 1], F32, tag="stepsb", name=f"st{t}")
                nc.vector.tensor_copy(step_sb, hd)
                nc.sync.dma_start(out=outT[:, d:d + 1], in_=step_sb)
                xcol = acts.tile([64, 1], F32, tag="xcol", name=f"xc{t}")
                nc.vector.tensor_copy(xcol, hd)
            nc.tensor.matmul(gps[3][TOP, :], ones64[0:1, :],
                             wt_t["r1"][0:1, 3 * 512:4 * 512],
                             start=True, stop=False, tile_position=(0, 0),
                             skip_group_check=True)
            if fb_u:
                for j in range(NCH):
                    nc.tensor.matmul(gps[j][BOT, :], ones64[0:1, :],
                                     rfb[0:1, j * 512:(j + 1) * 512],
                                     start=True, stop=False,
                                     tile_position=(0, 64),
                                     skip_group_check=True)

            # D: cell0@t rank-1 x update + split BOT chain (head-fed bottoms)
            if fb_t:
                gsum = fbpool.tile([64, G], F32, tag="gsum", name=f"gs{t}")
                fbt = fbpool.tile([64, G], F32, tag="fbt", name=f"fx{t}")
                gaps_b = []
                for j in range(NCH):
                    sl = slice(j * 512, (j + 1) * 512)
                    nc.vector.tensor_scalar_mul(fbt[:, sl], wbx[:, sl],
                                                xcol[:, 0:1])
                    nc.vector.tensor_add(gsum[:, sl], gps_prev[j][BOT, :],
                                         fbt[:, sl])
                    gaps_b.append(gsum[:, sl])
                emit_cell(gaps_b, ts_prev, BOT, c_prev2[BOT, :], c_prev[BOT, :])

            # E: w1h x16 (+ T_bot inject for head-fed bottoms)
            for idx in range(16):
                k, j = idx // KC, idx % KC
                nc.tensor.matmul(gps[j][TOP, :], h1T[:, k * B:(k + 1) * B],
                                 wt_t["w1h"][:, k, j * 512:(j + 1) * 512],
                                 start=False, stop=False, tile_position=(0, 0),
                                 skip_group_check=True)
                if idx == 15 and fb_t:
                    h0T = transpose4(ttB, 0, ts_prev["hsb"][BOT, :],
                                     ident[64:128, 64:128], "h0T", f"h0T_{t}")

            # F: zip w0(u) with w1i(t); both stop on k=3
            a_seq = [(j, k) for j in range(NCH) for k in range(KC)]
            b_seq = a_seq if have_bot else []
            for i in range(len(a_seq)):
                if i < len(b_seq):
                    j, k = b_seq[i]
                    nc.tensor.matmul(gps[j][BOT, :], h0T[:, k * B:(k + 1) * B],
                                     wt_b["w0"][:, k, j * 512:(j + 1) * 512],
                                     start=False, stop=k == KC - 1,
                                     tile_position=(0, 64),
                                     skip_group_check=True)
                j, k = a_seq[i]
                nc.tensor.matmul(gps[j][TOP, :], h0T[:, k * B:(k + 1) * B],
                                 wt_t["w1i"][:, k, j * 512:(j + 1) * 512],
                                 start=False, stop=k == KC - 1,
                                 tile_position=(0, 0), skip_group_check=True)

            # chains: stacked when cell0@u is encoder-fed, else TOP-only
            ts_cur = alloc_tset(f"t{t}")
            c_cur = states.tile([128, 512], F32, tag="c", name=f"c{u}")
            if enc_x:
                emit_cell([gps[j][:, :] for j in range(NCH)], ts_cur,
                          slice(0, 128), c_prev[:, :], c_cur[:, :])
            else:
                emit_cell([gps[j][TOP, :] for j in range(NCH)], ts_cur, TOP,
                          c_prev[TOP, :], c_cur[TOP, :])
            c_prev2, c_prev, ts_prev, gps_prev = c_prev, c_cur, ts_cur, gps

        # ---------------- tail: head(NT-1) ----------------
        ttA = tpool.tile([128, 256], F32, tag="ttA", name="ttA_end")
        ttB = tpool.tile([128, 256], F32, tag="ttB", name="ttB_end")
        h1T = transpose4(ttA, 0, ts_prev["hsb"][TOP, :], ident[0:64, 0:64],
                         "h1T", "h1T_end")
        hd = ttB[0:64, 255:256]
        nc.tensor.matmul(hd, ones64[0:1, :], headb[0:1, 0:1],
                         start=True, stop=False, skip_group_check=True)
        for k in range(KC):
            nc.tensor.matmul(hd, h1T[:, k * B:(k + 1) * B], headt[:, k:k + 1],
                             start=False, stop=k == KC - 1,
                             skip_group_check=True)
        step_sb = acts.tile([64, 1], F32, tag="stepsb", name="st_end")
        nc.vector.tensor_copy(step_sb, hd)
        nc.sync.dma_start(out=outT[:, n_dec - 1:n_dec], in_=step_sb)

    nc.compile()
    return nc


def _pack_weights_v2(inputs):
    import ml_dtypes
    f32 = np.float32
    ndt = ml_dtypes.bfloat16

    def wt_pack(w):  # [G, H] -> [128, KC, G]
        return np.ascontiguousarray(
            np.asarray(w, f32).T.reshape(KC, 128, G).transpose(1, 0, 2)).astype(ndt)

    head_b = float(np.asarray(inputs["head_b"], f32)[0])
    dcol = np.asarray(inputs["dec_Wih0"], f32)[:, 0]
    m = {
        "wt_e0": wt_pack(inputs["enc_Whh0"]),
        "wt_e1i": wt_pack(inputs["enc_Wih1"]),
        "wt_e1h": wt_pack(inputs["enc_Whh1"]),
        "wt_d0": wt_pack(inputs["dec_Whh0"]),
        "wt_d1i": wt_pack(inputs["dec_Wih1"]),
        "wt_d1h": wt_pack(inputs["dec_Whh1"]),
        "rows_e0": np.stack([np.asarray(inputs["enc_Wih0"], f32)[:, 0],
                             np.asarray(inputs["enc_b0"], f32)]).astype(ndt),
        "rows_e1": np.asarray(inputs["enc_b1"], f32)[None, :].astype(ndt),
        "rows_d0": np.stack([dcol, np.asarray(inputs["dec_b0"], f32)]).astype(ndt),
        "rows_d1": np.asarray(inputs["dec_b1"], f32)[None, :].astype(ndt),
        "rows_fb": (np.asarray(inputs["dec_b0"], f32) + head_b * dcol)[None, :].astype(ndt),
        "wbx": np.broadcast_to(dcol[None, :], (B, G)).astype(ndt),
        "headt": np.ascontiguousarray(
            np.asarray(inputs["head_W"], f32)[0].reshape(KC, 128).T).astype(ndt),
        "headb": np.full((1, B), head_b, ndt),
        "zeros": np.zeros((128, KC * B), ndt),
    }
    return {k: np.ascontiguousarray(v) for k, v in m.items()}

